# revision 5
# baseline (speedup 1.0000x reference)
"""Conditional BatchNorm1d (training mode) on 8 Trainium2 NeuronCores.

Class-streamed pipeline, v4 (experiment build):
  - all loads  : plain fp8 (half DMA bytes on the input side)
  - s1 fold1   : slots 0-7 on gpsimd (tensor_tensor fp8+fp8->fp16),
                 slots 8-15 on DVE (measures fp8 TT rate)
  - s1 tail    : DVE fold2 (fp16 TT 2x) + tensor_reduce
  - s2         : Act Square on fp8 + accum (1x, dtype-independent)
  - apply      : DVE tensor_scalar fp8 -> fp16 out (measures fp8 2x_2p)
  - stores fp16; Sqrt-first warmup so Square/Sqrt share one table set
"""
import numpy as np

N_CORES = 8
N = 500000
F = 128
C = 16
EPS = 1e-5

FPC = F // N_CORES           # 16 features per core
NBLK = N_CORES               # 8 row-blocks stacked on partitions
SLOT = 4096                  # columns per class slot
COLS = C * SLOT              # 65536 columns per core
HALF = SLOT // 2

_CACHE = {}


def _build():
    import concourse.bacc as bacc
    import concourse.bass as bass
    from concourse import mybir
    import concourse.tile as tile

    F32 = mybir.dt.float32
    F16 = mybir.dt.float16
    F8 = mybir.dt.float8e4
    AF = mybir.ActivationFunctionType
    ALU = mybir.AluOpType

    nc = bacc.Bacc("TRN2", target_bir_lowering=False, debug=False,
                   num_devices=N_CORES)
    xt = nc.dram_tensor("xt", [F, COLS], F8, kind="ExternalInput").ap()
    gt = nc.dram_tensor("gt", [F, C], F32, kind="ExternalInput").ap()
    bt = nc.dram_tensor("bt", [F, C], F32, kind="ExternalInput").ap()
    invn = nc.dram_tensor("invn", [F, C], F32, kind="ExternalInput").ap()
    amask = nc.dram_tensor("amask", [F, F], F32, kind="ExternalInput").ap()
    y = nc.dram_tensor("y", [F, COLS], F16, kind="ExternalOutput").ap()

    with tile.TileContext(nc) as tc:
        with (
            tc.tile_pool(name="const", bufs=1) as const,
            tc.tile_pool(name="x8p", bufs=16) as x8p,
            tc.tile_pool(name="x16p", bufs=4) as x16p,
            tc.tile_pool(name="dmp", bufs=2) as dmp,
            tc.tile_pool(name="tvp", bufs=2) as tvp,
            tc.tile_pool(name="tqp", bufs=2) as tqp,
            tc.tile_pool(name="smp", bufs=2) as smp,
            tc.tile_pool(name="ps", bufs=1, space="PSUM") as psp,
        ):
            # ---- constants + Act table warmup (Sqrt first: one table set) ----
            eps_sb = const.tile([F, 1], F32)
            nc.vector.memset(eps_sb[:], EPS)
            warm_sb = const.tile([F, 1], F32)
            nc.scalar.activation(out=warm_sb[:], in_=eps_sb[:], func=AF.Sqrt,
                                 bias=eps_sb[:])

            gt_sb = const.tile([F, C], F32)
            nc.sync.dma_start(out=gt_sb[:], in_=gt)
            bt_sb = const.tile([F, C], F32)
            nc.sync.dma_start(out=bt_sb[:], in_=bt)
            invn_sb = const.tile([F, C], F32)
            nc.sync.dma_start(out=invn_sb[:], in_=invn)
            amask_sb = const.tile([F, F], F32)
            nc.sync.dma_start(out=amask_sb[:], in_=amask)

            st1 = const.tile([F, C], F32)
            st2 = const.tile([F, C], F32)
            scale = const.tile([F, C], F32)
            shift = const.tile([F, C], F32)
            psum1 = psp.tile([F, C], F32)
            psum2 = psp.tile([F, C], F32)

            # ---- slot loads: plain fp8, all resident ----
            xg8 = []
            for s in range(C):
                x8_s = x8p.tile([F, SLOT], F8, tag="x8", name=f"x8_{s}")
                xg8.append(x8_s)
                src = bass.AP(tensor=xt.tensor, offset=s * SLOT,
                              ap=[[COLS, F], [1, SLOT]])
                nc.sync.dma_start(out=x8_s[:], in_=src)

            # ---- streamed per-slot pipeline, chains per 2 slots ----
            for g in range(C // 2):
                for s in (2 * g, 2 * g + 1):
                    x8_s = xg8[s]
                    tv = tvp.tile([F, HALF], F16, tag="tv", name=f"tv_{s}")
                    eng = nc.gpsimd if s < 8 else nc.vector
                    eng.tensor_tensor(out=tv[:], in0=x8_s[:, 0:HALF],
                                      in1=x8_s[:, HALF:SLOT], op=ALU.add)
                    tq = tqp.tile([F, HALF // 2], F16, tag="tq",
                                  name=f"tq_{s}")
                    nc.vector.tensor_tensor(
                        out=tq[:], in0=tv[:, 0:HALF // 2],
                        in1=tv[:, HALF // 2:HALF], op=ALU.add)
                    nc.vector.tensor_reduce(
                        out=st1[:, s:s + 1], in_=tq[:],
                        axis=mybir.AxisListType.X, op=ALU.add)
                    # s2 on Act, fp8 src
                    dm = dmp.tile([F, SLOT], F8, tag="dm", name=f"dm_{s}")
                    nc.scalar.activation(out=dm[:], in_=x8_s[:],
                                         func=AF.Square,
                                         accum_out=st2[:, s:s + 1])

                # fold the 8 row-blocks for both classes of this group
                c0, c1 = 2 * g, 2 * g + 2
                nc.tensor.matmul(out=psum1[:, c0:c1], lhsT=amask_sb[:],
                                 rhs=st1[:, c0:c1], start=True, stop=True)
                nc.tensor.matmul(out=psum2[:, c0:c1], lhsT=amask_sb[:],
                                 rhs=st2[:, c0:c1], start=True, stop=True)

                # ---- per-group chain: stats -> scale/shift ----
                mg = smp.tile([F, 2], F32, tag="mg", name=f"mg_{g}")
                nc.vector.tensor_tensor(out=mg[:], in0=psum1[:, c0:c1],
                                        in1=invn_sb[:, c0:c1], op=ALU.mult)
                eg = smp.tile([F, 2], F32, tag="eg", name=f"eg_{g}")
                nc.vector.tensor_tensor(out=eg[:], in0=psum2[:, c0:c1],
                                        in1=invn_sb[:, c0:c1], op=ALU.mult)
                vg = smp.tile([F, 2], F32, tag="vg", name=f"vg_{g}")
                nc.vector.tensor_tensor(out=vg[:], in0=mg[:], in1=mg[:],
                                        op=ALU.mult)
                nc.vector.tensor_tensor(out=vg[:], in0=eg[:], in1=vg[:],
                                        op=ALU.subtract)
                sg = smp.tile([F, 2], F32, tag="sg", name=f"sg_{g}")
                nc.scalar.activation(out=sg[:], in_=vg[:], func=AF.Sqrt,
                                     bias=eps_sb[:])
                ig = smp.tile([F, 2], F32, tag="ig", name=f"ig_{g}")
                nc.vector.reciprocal(out=ig[:], in_=sg[:])
                nc.vector.tensor_tensor(out=scale[:, c0:c1],
                                        in0=gt_sb[:, c0:c1], in1=ig[:],
                                        op=ALU.mult)
                tg = smp.tile([F, 2], F32, tag="tg", name=f"tg_{g}")
                nc.vector.tensor_tensor(out=tg[:], in0=mg[:],
                                        in1=scale[:, c0:c1], op=ALU.mult)
                nc.vector.tensor_tensor(out=shift[:, c0:c1],
                                        in0=bt_sb[:, c0:c1], in1=tg[:],
                                        op=ALU.subtract)

                # ---- apply (fp8 -> fp16) + store ----
                for s in (2 * g, 2 * g + 1):
                    x16_s = x16p.tile([F, SLOT], F16, tag="x16",
                                      name=f"x16_{s}")
                    nc.vector.tensor_scalar(
                        out=x16_s[:], in0=xg8[s][:],
                        scalar1=scale[:, s:s + 1], scalar2=shift[:, s:s + 1],
                        op0=ALU.mult, op1=ALU.add)
                    dst = bass.AP(tensor=y.tensor, offset=s * SLOT,
                                  ap=[[COLS, F], [1, SLOT]])
                    nc.scalar.dma_start(out=dst, in_=x16_s[:])
    nc.finalize()
    return nc


def _get_nc():
    if "nc" not in _CACHE:
        _CACHE["nc"] = _build()
    return _CACHE["nc"]


def _numpy_fallback(x, labels, gamma, beta):
    counts = np.maximum(np.bincount(labels, minlength=C), 1).astype(np.float32)
    s1 = np.zeros((C, F), np.float32)
    s2 = np.zeros((C, F), np.float32)
    for c in range(C):
        m = labels == c
        s1[c] = x[m].sum(0)
        s2[c] = (x[m] * x[m]).sum(0)
    mean = s1 / counts[:, None]
    var = s2 / counts[:, None] - mean * mean
    istd = 1.0 / np.sqrt(var + EPS)
    scale = gamma * istd
    shift = beta - mean * scale
    return x * scale[labels] + shift[labels]


def kernel(x, labels, gamma, beta):
    import ml_dtypes
    from concourse.bass_utils import run_bass_kernel_spmd

    x = np.ascontiguousarray(np.asarray(x, dtype=np.float32))
    labels_np = np.asarray(labels).astype(np.int64)
    gamma = np.ascontiguousarray(np.asarray(gamma, dtype=np.float32))
    beta = np.ascontiguousarray(np.asarray(beta, dtype=np.float32))

    counts = np.bincount(labels_np, minlength=C)
    if int(counts.max()) > NBLK * SLOT:
        return _numpy_fallback(x, labels_np, gamma, beta)

    order = np.argsort(labels_np, kind="stable")
    starts = np.concatenate([[0], np.cumsum(counts)])
    chunks = [np.array_split(order[starts[c]:starts[c + 1]], NBLK)
              for c in range(C)]

    invn = (1.0 / np.maximum(counts, 1)).astype(np.float32)
    invn_b = np.ascontiguousarray(np.broadcast_to(invn, (F, C)))
    amask = np.tile(np.eye(FPC, dtype=np.float32), (NBLK, NBLK))
    amask = np.ascontiguousarray(amask)

    xh8 = np.clip(x, -240.0, 240.0).astype(ml_dtypes.float8_e4m3)
    blocks8 = []
    for b in range(NBLK):
        xb8 = np.zeros((F, COLS), dtype=ml_dtypes.float8_e4m3)
        for c in range(C):
            rows = chunks[c][b]
            xb8[:, c * SLOT:c * SLOT + len(rows)] = xh8[rows].T
        blocks8.append(xb8)

    in_maps = []
    for k in range(N_CORES):
        fsl = slice(k * FPC, (k + 1) * FPC)
        xt_k = np.concatenate([blocks8[b][fsl] for b in range(NBLK)], axis=0)
        gt_k = np.ascontiguousarray(
            np.tile(gamma.T[fsl], (NBLK, 1)))          # [(b,f), c]
        bt_k = np.ascontiguousarray(np.tile(beta.T[fsl], (NBLK, 1)))
        in_maps.append({"xt": np.ascontiguousarray(xt_k), "gt": gt_k,
                        "bt": bt_k, "invn": invn_b, "amask": amask})

    nc = _get_nc()
    res = run_bass_kernel_spmd(nc, in_maps, core_ids=list(range(N_CORES)),
                               **_CACHE.get("run_kwargs", {}))
    _CACHE["last_results"] = res

    y = np.empty((N, F), dtype=np.float32)
    for k in range(N_CORES):
        yk = res.results[k]["y"]
        fsl = slice(k * FPC, (k + 1) * FPC)
        for b in range(NBLK):
            ybf = yk[b * FPC:(b + 1) * FPC]
            for c in range(C):
                rows = chunks[c][b]
                y[rows, fsl] = ybf[:, c * SLOT:c * SLOT + len(rows)].T
    return y
